# revision 38
# baseline (speedup 1.0000x reference)
"""Trainium2 Bass kernel for the LIF dense layer (spike output only).

The reference computes
    P_n   = quant8(alpha*P + Q)            (grid 1/128, round-half-even)
    U     = P_n @ quant8(W) + quant8(b) - S
    S_n   = (U > 0.4)
``input_t`` and ``R`` never influence the output (Q_n/U_q are dead,
gamma == 0), so they are never loaded.

All quantized operands are 8-bit integers scaled by 1/128, hence exactly
representable in bf16, and every partial matmul sum is a multiple of 2^-14
below 2^24 -> bf16 matmul with fp32 PSUM accumulation is bit-exact vs the
fp32 reference einsum.  Rounding uses the fp32 magic-number trick
(x + 1.5*2^16) - 1.5*2^16 == round-to-nearest-even onto the 1/128 grid.
The spike compare is folded as  U - S > thr  <=>  U > thr + S  (exact for
S in {0,1}: U is on the 2^-14 grid, |delta| >> ulp(1.4)).

The contraction dim must sit on SBUF partitions for the matmul, so the
quantized activations are transposed on the TensorEngine (128x128 identity
matmuls into PSUM, evicted to SBUF by the DVE).  An earlier version used
``dma_start_transpose`` (DMA xbar) for this, which runs ~450us per 128x128
bf16 tile on hardware and dominated the kernel (~59ms); the PE transpose is
~0.1us per tile.

S and the spike output are exactly 0/1, so they are staged through HBM as
uint8 (host converts), cutting DMA traffic from 32 MiB to 20.5 MiB per
core; P/Q stay fp32 because the quantizer's rounding decisions need full
precision.  Work is spread so no engine exceeds the ~60us DMA floor:
ACT scale+round, DVE add+evict+compare, Pool clamp+threshold, PE
transposes+matmuls.

Sharding: pure data parallel over the batch dim, 4096 rows per core on 8
NeuronCores; the [512,512] weights / bias are quantized host-side (exact
replication of the reference quantizer) and replicated.
"""

import sys

import numpy as np

sys.path.insert(0, "/opt/trn_rl_repo")

import ml_dtypes

B, IN, OUT = 32768, 512, 512
NCORES = 8
BL = B // NCORES            # rows per core
PART = 128                  # SBUF partitions
KCH = IN // PART            # contraction chunks of 128
G = 2                       # 128-row tiles per DMA super-tile
# exp(-dt/tau_mem) as computed by XLA fp32 (1 ulp above numpy's expf)
ALPHA = float(np.array(1062312023, np.uint32).view(np.float32))
MAGIC = 98304.0             # 1.5*2^16: fp32 +/- rounds to multiples of 2^-7
QMAX = 127.0 / 128.0
THR = 0.4


def build_nc(bl=BL, g=G, enable_asserts=False):
    import concourse.bass as bass
    import concourse.bacc as bacc
    import concourse.mybir as mybir
    from concourse import tile
    from concourse.masks import make_identity

    OP = mybir.AluOpType
    AF = mybir.ActivationFunctionType
    dt = mybir.dt
    ts = bass.ts

    ntiles = bl // PART
    assert ntiles % g == 0
    nsuper = ntiles // g

    # Bacc (not plain Bass): its compile() splits multi-sem waits into
    # event semaphores -- TRN2 allows one wait per instruction.
    nc = bacc.Bacc(
        "TRN2",
        target_bir_lowering=False,
        debug=False,
        enable_asserts=enable_asserts,
        num_devices=NCORES,
    )
    p_d = nc.dram_tensor("p", [bl, IN], dt.float32, kind="ExternalInput").ap()
    q_d = nc.dram_tensor("q", [bl, IN], dt.float32, kind="ExternalInput").ap()
    s_d = nc.dram_tensor("s", [bl, OUT], dt.uint8, kind="ExternalInput").ap()
    w_d = nc.dram_tensor("w", [IN, OUT], dt.bfloat16, kind="ExternalInput").ap()
    b_d = nc.dram_tensor("bq", [1, OUT], dt.bfloat16, kind="ExternalInput").ap()
    o_d = nc.dram_tensor("o", [bl, OUT], dt.uint8, kind="ExternalOutput").ap()

    # partition-major views: one DMA moves [128, g, 512] per super-tile
    pv = p_d.rearrange("(n p) i -> p n i", p=PART)
    qv = q_d.rearrange("(n p) i -> p n i", p=PART)
    sv = s_d.rearrange("(n p) i -> p n i", p=PART)
    ov = o_d.rearrange("(n p) i -> p n i", p=PART)
    wv = w_d.rearrange("(k p) o -> p k o", p=PART)

    with tile.TileContext(nc) as tc:
        with (
            tc.tile_pool(name="const", bufs=1) as cpool,
            tc.tile_pool(name="io", bufs=10) as iop,
            tc.tile_pool(name="work", bufs=4) as wkp,
            tc.tile_pool(name="thr", bufs=8) as thrp,
            tc.tile_pool(name="lhs", bufs=5) as lhp,
            tc.tile_pool(name="out", bufs=12) as outp,
            tc.tile_pool(name="psT", bufs=4, space="PSUM") as psT,
            tc.tile_pool(name="psU", bufs=3, space="PSUM") as psU,
        ):
            w_sb = cpool.tile([PART, KCH, OUT], dt.bfloat16)
            nc.sync.dma_start(out=w_sb[:], in_=wv[:])
            bq_sb = cpool.tile([1, OUT], dt.bfloat16)
            nc.sync.dma_start(out=bq_sb[:], in_=b_d[:])
            ones_sb = cpool.tile([1, PART], dt.bfloat16)
            nc.vector.memset(ones_sb[:], 1.0)
            magic_p = cpool.tile([PART, 1], dt.float32)
            nc.vector.memset(magic_p[:], MAGIC)
            magic_n = cpool.tile([PART, 1], dt.float32)
            nc.vector.memset(magic_n[:], -MAGIC)
            ident = cpool.tile([PART, PART], dt.bfloat16)
            make_identity(nc, ident[:])

            # Fully stage-skewed software pipeline.  Every engine's queue is
            # emitted so that an instruction's cross-engine dependencies were
            # produced 1-2 groups earlier: an op that waits mid-queue
            # head-of-line blocks every later op on that engine, so stage k
            # of group i is emitted next to stage k+1 of group i-1.
            OUT_SKEW = 12
            groups = [(si, 0, g) for si in range(nsuper - 2)]
            for si in (nsuper - 2, nsuper - 1):  # drain at finer grain
                groups += [(si, j, 1) for j in range(g)]
            ngrp = len(groups)

            st_dma = {}    # gi -> (p_t, q_t, s_t)
            st_x = {}      # gi -> x_t
            st_q8 = {}     # gi -> (q8_t, thr_t)
            st_mm = []     # row-tile queue: (q8T_sb, thr_slice, sp_slice, done)
            pend_out = []

            def s0_load(gi):
                si, j0, gn = groups[gi]
                p_t = iop.tile([PART, gn, IN], dt.float32, tag="p")
                q_t = iop.tile([PART, gn, IN], dt.float32, tag="q")
                s_t = iop.tile([PART, gn, OUT], dt.uint8, tag="s")
                tsl = slice(si * g + j0, si * g + j0 + gn)
                nc.sync.dma_start(out=p_t[:], in_=pv[:, tsl, :])
                nc.sync.dma_start(out=q_t[:], in_=qv[:, tsl, :])
                nc.sync.dma_start(out=s_t[:], in_=sv[:, tsl, :])
                st_dma[gi] = (p_t, q_t, s_t)

            def s1_scale(gi):
                _, _, gn = groups[gi]
                p_t, _, _ = st_dma[gi]
                x_t = wkp.tile([PART, gn, IN], dt.float32, tag="x")
                nc.scalar.activation(x_t[:], p_t[:], AF.Copy, scale=ALPHA)
                st_x[gi] = x_t

            def s2_addclamp(gi):
                # x = clip(alpha*P + Q, +/-127/128); clamping before rounding
                # matches the reference and makes a post-round clamp
                # unnecessary (|round(x)| <= 127/128)
                _, _, gn = groups[gi]
                _, q_t, s_t = st_dma[gi]
                x_t = st_x[gi]
                nc.vector.tensor_add(x_t[:], x_t[:], q_t[:])
                nc.gpsimd.tensor_scalar(
                    out=x_t[:], in0=x_t[:], scalar1=QMAX, scalar2=-QMAX,
                    op0=OP.min, op1=OP.max,
                )
                # thr + S on GPSIMD (u8 -> f32), off the DVE/ACT chain
                thr_t = thrp.tile([PART, gn, OUT], dt.float32, tag="thr")
                nc.gpsimd.tensor_scalar_add(thr_t[:], s_t[:], THR)
                st_q8[gi] = (None, thr_t)

            def s3_round(gi):
                # round-half-even onto the 1/128 grid: +MAGIC rounds in fp32,
                # -MAGIC is exact (Sterbenz) and narrows to bf16 (also
                # exact); both on ACT, back to back
                _, _, gn = groups[gi]
                x_t = st_x.pop(gi)
                q8_t = wkp.tile([PART, gn, IN], dt.bfloat16, tag="q8")
                nc.scalar.activation(x_t[:], x_t[:], AF.Identity, bias=magic_p[:])
                nc.scalar.activation(q8_t[:], x_t[:], AF.Identity, bias=magic_n[:])
                st_q8[gi] = (q8_t, st_q8[gi][1])

            def s4_transpose(gi):
                # PE transpose: contract dim onto partitions; DVE evicts the
                # bf16 PSUM tile to SBUF for use as the matmul stationary
                si, j0, gn = groups[gi]
                q8_t, thr_t = st_q8.pop(gi)
                # one output tile + DMA per group: an SP DMA trigger costs
                # ~0.65us of sequencer issue time regardless of size
                sp_t = outp.tile([PART, gn, OUT], dt.uint8, tag="sp")
                tsl = slice(si * g + j0, si * g + j0 + gn)
                for j in range(gn):
                    q8T_ps = psT.tile([PART, KCH, PART], dt.bfloat16, tag="q8T")
                    for k in range(KCH):
                        nc.tensor.transpose(
                            q8T_ps[:, k, :], q8_t[:, j, ts(k, PART)], ident[:]
                        )
                    q8T_sb = lhp.tile([PART, KCH, PART], dt.bfloat16, tag="q8Ts")
                    nc.vector.tensor_copy(q8T_sb[:], q8T_ps[:])
                    st_mm.append((
                        q8T_sb, thr_t[:, j, :], sp_t[:, j, :],
                        (ov[:, tsl, :], sp_t) if j == gn - 1 else None,
                    ))

            def s5_matmul():
                q8T_sb, thr_slice, sp_slice, done = st_mm.pop(0)
                u_ps = psU.tile([PART, OUT], dt.float32, tag="u")
                for k in range(KCH):
                    nc.tensor.matmul(
                        u_ps[:],
                        lhsT=q8T_sb[:, k, :],
                        rhs=w_sb[:, k, :],
                        start=(k == 0),
                        stop=False,
                    )
                # bias as a K=1 accumulation: ones.T @ bq
                nc.tensor.matmul(
                    u_ps[:], lhsT=ones_sb[:], rhs=bq_sb[:],
                    start=False, stop=True,
                )
                # spike = U > thr + S  (exact <=> (U - S) > thr); 0/1 as u8
                nc.vector.tensor_tensor(sp_slice, u_ps[:], thr_slice, OP.is_gt)
                if done is not None:
                    pend_out.append(done)
                    if len(pend_out) > OUT_SKEW:
                        tv, sp = pend_out.pop(0)
                        nc.sync.dma_start(out=tv, in_=sp[:])

            for i in range(ngrp + 4):
                if i < ngrp:
                    s0_load(i)
                    s1_scale(i)
                if 0 <= i - 1 < ngrp:
                    s2_addclamp(i - 1)
                if 0 <= i - 2 < ngrp:
                    s3_round(i - 2)
                if 0 <= i - 3 < ngrp:
                    s4_transpose(i - 3)
                    # keep one row-tile of skew between transposes and
                    # matmuls on the PE queue
                    while len(st_mm) > 1:
                        s5_matmul()
            while st_mm:
                s5_matmul()
            for tv, sp in pend_out:
                nc.sync.dma_start(out=tv, in_=sp[:])
    nc.finalize()  # Bacc.compile(): splits multi-sem waits (TRN2 1-wait rule)
    return nc


def _quant_host(x):
    """Exact replica of the reference quant_ste forward pass (fp32)."""
    x = np.asarray(x, np.float32)
    d = np.float32(1.0) / np.float32(128.0)
    y = np.clip(x, np.float32(-1.0) + d, np.float32(1.0) - d)
    y = y * np.float32(128.0)
    y = np.round(y)  # round-half-even, same as jnp.round
    return (y / np.float32(128.0)).astype(np.float32)


_cache = {}


def kernel(**inputs):
    from concourse.bass_utils import run_bass_kernel_spmd

    P = np.ascontiguousarray(np.asarray(inputs["P"], np.float32))
    Q = np.ascontiguousarray(np.asarray(inputs["Q"], np.float32))
    # S is exactly 0.0/1.0; stage it (and the 0/1 spike output) as uint8 to
    # cut HBM traffic -- the on-chip math still runs in fp32
    S = np.ascontiguousarray(np.asarray(inputs["S"], np.float32).astype(np.uint8))
    W = np.asarray(inputs["weights"], np.float32)
    bias = np.asarray(inputs["bias"], np.float32)

    wq = _quant_host(W).astype(ml_dtypes.bfloat16)
    bq = _quant_host(bias).reshape(1, OUT).astype(ml_dtypes.bfloat16)

    if "nc" not in _cache:
        _cache["nc"] = build_nc()
    nc = _cache["nc"]

    in_maps = []
    for c in range(NCORES):
        sl = slice(c * BL, (c + 1) * BL)
        in_maps.append({"p": P[sl], "q": Q[sl], "s": S[sl], "w": wq, "bq": bq})
    res = run_bass_kernel_spmd(nc, in_maps, list(range(NCORES)))
    _cache["last"] = res  # exec_time_ns etc. when tracing is enabled
    out = np.concatenate([res.results[c]["o"] for c in range(NCORES)], axis=0)
    return np.ascontiguousarray(out.astype(np.float32))


# revision 43
# speedup vs baseline: 1.0003x; 1.0003x over previous
"""Trainium2 Bass kernel for the LIF dense layer (spike output only).

The reference computes
    P_n   = quant8(alpha*P + Q)            (grid 1/128, round-half-even)
    U     = P_n @ quant8(W) + quant8(b) - S
    S_n   = (U > 0.4)
``input_t`` and ``R`` never influence the output (Q_n/U_q are dead,
gamma == 0), so they are never loaded.

All quantized operands are 8-bit integers scaled by 1/128, hence exactly
representable in bf16, and every partial matmul sum is a multiple of 2^-14
below 2^24 -> bf16 matmul with fp32 PSUM accumulation is bit-exact vs the
fp32 reference einsum.  Rounding uses the fp32 magic-number trick
(x + 1.5*2^16) - 1.5*2^16 == round-to-nearest-even onto the 1/128 grid.
The spike compare is folded as  U - S > thr  <=>  U > thr + S  (exact for
S in {0,1}: U is on the 2^-14 grid, |delta| >> ulp(1.4)).

The contraction dim must sit on SBUF partitions for the matmul, so the
quantized activations are transposed on the TensorEngine (128x128 identity
matmuls into PSUM, evicted to SBUF by the DVE).  An earlier version used
``dma_start_transpose`` (DMA xbar) for this, which runs ~450us per 128x128
bf16 tile on hardware and dominated the kernel (~59ms); the PE transpose is
~0.1us per tile.

S and the spike output are exactly 0/1, so they are staged through HBM as
uint8 (host converts), cutting DMA traffic from 32 MiB to 20.5 MiB per
core; P/Q stay fp32 because the quantizer's rounding decisions need full
precision.  Work is spread so no engine exceeds the ~60us DMA floor:
ACT scale+round, DVE add+evict+compare, Pool clamp+threshold, PE
transposes+matmuls.

Sharding: pure data parallel over the batch dim, 4096 rows per core on 8
NeuronCores; the [512,512] weights / bias are quantized host-side (exact
replication of the reference quantizer) and replicated.
"""

import sys

import numpy as np

sys.path.insert(0, "/opt/trn_rl_repo")

import ml_dtypes

B, IN, OUT = 32768, 512, 512
NCORES = 8
BL = B // NCORES            # rows per core
PART = 128                  # SBUF partitions
KCH = IN // PART            # contraction chunks of 128
G = 2                       # 128-row tiles per DMA super-tile
# exp(-dt/tau_mem) as computed by XLA fp32 (1 ulp above numpy's expf)
ALPHA = float(np.array(1062312023, np.uint32).view(np.float32))
MAGIC = 98304.0             # 1.5*2^16: fp32 +/- rounds to multiples of 2^-7
QMAX = 127.0 / 128.0
THR = 0.4


def build_nc(bl=BL, g=G, enable_asserts=False):
    import concourse.bass as bass
    import concourse.bacc as bacc
    import concourse.mybir as mybir
    from concourse import tile
    from concourse.masks import make_identity

    OP = mybir.AluOpType
    AF = mybir.ActivationFunctionType
    dt = mybir.dt
    ts = bass.ts

    ntiles = bl // PART
    assert ntiles % g == 0
    nsuper = ntiles // g

    # Bacc (not plain Bass): its compile() splits multi-sem waits into
    # event semaphores -- TRN2 allows one wait per instruction.
    nc = bacc.Bacc(
        "TRN2",
        target_bir_lowering=False,
        debug=False,
        enable_asserts=enable_asserts,
        num_devices=NCORES,
    )
    p_d = nc.dram_tensor("p", [bl, IN], dt.float32, kind="ExternalInput").ap()
    q_d = nc.dram_tensor("q", [bl, IN], dt.float32, kind="ExternalInput").ap()
    s_d = nc.dram_tensor("s", [bl, OUT], dt.uint8, kind="ExternalInput").ap()
    w_d = nc.dram_tensor("w", [IN, OUT], dt.bfloat16, kind="ExternalInput").ap()
    b_d = nc.dram_tensor("bq", [1, OUT], dt.bfloat16, kind="ExternalInput").ap()
    o_d = nc.dram_tensor("o", [bl, OUT], dt.uint8, kind="ExternalOutput").ap()

    # partition-major views: one DMA moves [128, g, 512] per super-tile
    pv = p_d.rearrange("(n p) i -> p n i", p=PART)
    qv = q_d.rearrange("(n p) i -> p n i", p=PART)
    sv = s_d.rearrange("(n p) i -> p n i", p=PART)
    ov = o_d.rearrange("(n p) i -> p n i", p=PART)
    wv = w_d.rearrange("(k p) o -> p k o", p=PART)

    with tile.TileContext(nc) as tc:
        with (
            tc.tile_pool(name="const", bufs=1) as cpool,
            tc.tile_pool(name="io", bufs=10) as iop,
            tc.tile_pool(name="work", bufs=4) as wkp,
            tc.tile_pool(name="thr", bufs=8) as thrp,
            tc.tile_pool(name="lhs", bufs=5) as lhp,
            tc.tile_pool(name="out", bufs=12) as outp,
            tc.tile_pool(name="psT", bufs=4, space="PSUM") as psT,
            tc.tile_pool(name="psU", bufs=3, space="PSUM") as psU,
        ):
            w_sb = cpool.tile([PART, KCH, OUT], dt.bfloat16)
            nc.sync.dma_start(out=w_sb[:], in_=wv[:])
            bq_sb = cpool.tile([1, OUT], dt.bfloat16)
            nc.sync.dma_start(out=bq_sb[:], in_=b_d[:])
            ones_sb = cpool.tile([1, PART], dt.bfloat16)
            nc.vector.memset(ones_sb[:], 1.0)
            magic_p = cpool.tile([PART, 1], dt.float32)
            nc.vector.memset(magic_p[:], MAGIC)
            magic_n = cpool.tile([PART, 1], dt.float32)
            nc.vector.memset(magic_n[:], -MAGIC)
            ident = cpool.tile([PART, PART], dt.bfloat16)
            make_identity(nc, ident[:])

            # Fully stage-skewed software pipeline.  Every engine's queue is
            # emitted so that an instruction's cross-engine dependencies were
            # produced 1-2 groups earlier: an op that waits mid-queue
            # head-of-line blocks every later op on that engine, so stage k
            # of group i is emitted next to stage k+1 of group i-1.
            OUT_SKEW = 12
            groups = [(si, 0, g) for si in range(nsuper - 2)]
            for si in (nsuper - 2, nsuper - 1):  # drain at finer grain
                groups += [(si, j, 1) for j in range(g)]
            ngrp = len(groups)

            st_dma = {}    # gi -> (p_t, q_t, s_t)
            st_x = {}      # gi -> x_t
            st_q8 = {}     # gi -> (q8_t, thr_t)
            st_mm = []     # row-tile queue: (q8T_sb, thr_slice, sp_slice, done)
            pend_out = []

            def s0_load(gi):
                si, j0, gn = groups[gi]
                p_t = iop.tile([PART, gn, IN], dt.float32, tag="p")
                q_t = iop.tile([PART, gn, IN], dt.float32, tag="q")
                s_t = iop.tile([PART, gn, OUT], dt.uint8, tag="s")
                tsl = slice(si * g + j0, si * g + j0 + gn)
                nc.sync.dma_start(out=p_t[:], in_=pv[:, tsl, :])
                nc.sync.dma_start(out=q_t[:], in_=qv[:, tsl, :])
                nc.sync.dma_start(out=s_t[:], in_=sv[:, tsl, :])
                st_dma[gi] = (p_t, q_t, s_t)

            def s1_scale(gi):
                _, _, gn = groups[gi]
                p_t, _, _ = st_dma[gi]
                x_t = wkp.tile([PART, gn, IN], dt.float32, tag="x")
                nc.scalar.activation(x_t[:], p_t[:], AF.Copy, scale=ALPHA)
                st_x[gi] = x_t

            def s2_addclamp(gi):
                # x = clip(alpha*P + Q, +/-127/128); clamping before rounding
                # matches the reference and makes a post-round clamp
                # unnecessary (|round(x)| <= 127/128)
                _, _, gn = groups[gi]
                _, q_t, s_t = st_dma[gi]
                x_t = st_x[gi]
                nc.vector.tensor_add(x_t[:], x_t[:], q_t[:])
                nc.gpsimd.tensor_scalar(
                    out=x_t[:], in0=x_t[:], scalar1=QMAX, scalar2=-QMAX,
                    op0=OP.min, op1=OP.max,
                )
                # thr + S on GPSIMD (u8 -> f32), off the DVE/ACT chain
                thr_t = thrp.tile([PART, gn, OUT], dt.float32, tag="thr")
                nc.gpsimd.tensor_scalar_add(thr_t[:], s_t[:], THR)
                st_q8[gi] = (None, thr_t)

            def s3_round(gi):
                # round-half-even onto the 1/128 grid: +MAGIC rounds in fp32,
                # -MAGIC is exact (Sterbenz) and narrows to bf16 (also
                # exact); both on ACT, back to back
                _, _, gn = groups[gi]
                x_t = st_x.pop(gi)
                q8_t = wkp.tile([PART, gn, IN], dt.bfloat16, tag="q8")
                nc.scalar.activation(x_t[:], x_t[:], AF.Identity, bias=magic_p[:])
                nc.scalar.activation(q8_t[:], x_t[:], AF.Identity, bias=magic_n[:])
                st_q8[gi] = (q8_t, st_q8[gi][1])

            def s4_transpose(gi):
                # PE transpose: contract dim onto partitions; DVE evicts the
                # bf16 PSUM tile to SBUF for use as the matmul stationary
                si, j0, gn = groups[gi]
                q8_t, thr_t = st_q8.pop(gi)
                # one output tile + DMA per group: an SP DMA trigger costs
                # ~0.65us of sequencer issue time regardless of size
                sp_t = outp.tile([PART, gn, OUT], dt.uint8, tag="sp")
                tsl = slice(si * g + j0, si * g + j0 + gn)
                for j in range(gn):
                    q8T_ps = psT.tile([PART, KCH, PART], dt.bfloat16, tag="q8T")
                    for k in range(KCH):
                        nc.tensor.transpose(
                            q8T_ps[:, k, :], q8_t[:, j, ts(k, PART)], ident[:]
                        )
                    q8T_sb = lhp.tile([PART, KCH, PART], dt.bfloat16, tag="q8Ts")
                    nc.vector.tensor_copy(q8T_sb[:], q8T_ps[:])
                    st_mm.append((
                        q8T_sb, thr_t[:, j, :], sp_t[:, j, :],
                        (ov[:, tsl, :], sp_t) if j == gn - 1 else None,
                    ))

            def s5_matmul():
                q8T_sb, thr_slice, sp_slice, done = st_mm.pop(0)
                u_ps = psU.tile([PART, OUT], dt.float32, tag="u")
                for k in range(KCH):
                    nc.tensor.matmul(
                        u_ps[:],
                        lhsT=q8T_sb[:, k, :],
                        rhs=w_sb[:, k, :],
                        start=(k == 0),
                        stop=False,
                    )
                # bias as a K=1 accumulation: ones.T @ bq
                nc.tensor.matmul(
                    u_ps[:], lhsT=ones_sb[:], rhs=bq_sb[:],
                    start=False, stop=True,
                )
                # spike = U > thr + S  (exact <=> (U - S) > thr); 0/1 as u8
                nc.vector.tensor_tensor(sp_slice, u_ps[:], thr_slice, OP.is_gt)
                if done is not None:
                    pend_out.append(done)
                    if len(pend_out) > OUT_SKEW:
                        tv, sp = pend_out.pop(0)
                        nc.sync.dma_start(out=tv, in_=sp[:])

            for i in range(ngrp + 4):
                if i < ngrp:
                    s0_load(i)
                    s1_scale(i)
                if 0 <= i - 1 < ngrp:
                    s2_addclamp(i - 1)
                if 0 <= i - 2 < ngrp:
                    s3_round(i - 2)
                if 0 <= i - 3 < ngrp:
                    s4_transpose(i - 3)
                    # keep two row-tiles of skew between transposes and
                    # matmuls on the PE queue
                    while len(st_mm) > 2:
                        s5_matmul()
            while st_mm:
                s5_matmul()
            for tv, sp in pend_out:
                nc.sync.dma_start(out=tv, in_=sp[:])
    nc.finalize()  # Bacc.compile(): splits multi-sem waits (TRN2 1-wait rule)
    return nc


def _quant_host(x):
    """Exact replica of the reference quant_ste forward pass (fp32)."""
    x = np.asarray(x, np.float32)
    d = np.float32(1.0) / np.float32(128.0)
    y = np.clip(x, np.float32(-1.0) + d, np.float32(1.0) - d)
    y = y * np.float32(128.0)
    y = np.round(y)  # round-half-even, same as jnp.round
    return (y / np.float32(128.0)).astype(np.float32)


_cache = {}


def kernel(**inputs):
    from concourse.bass_utils import run_bass_kernel_spmd

    P = np.ascontiguousarray(np.asarray(inputs["P"], np.float32))
    Q = np.ascontiguousarray(np.asarray(inputs["Q"], np.float32))
    # S is exactly 0.0/1.0; stage it (and the 0/1 spike output) as uint8 to
    # cut HBM traffic -- the on-chip math still runs in fp32
    S = np.ascontiguousarray(np.asarray(inputs["S"], np.float32).astype(np.uint8))
    W = np.asarray(inputs["weights"], np.float32)
    bias = np.asarray(inputs["bias"], np.float32)

    wq = _quant_host(W).astype(ml_dtypes.bfloat16)
    bq = _quant_host(bias).reshape(1, OUT).astype(ml_dtypes.bfloat16)

    if "nc" not in _cache:
        _cache["nc"] = build_nc()
    nc = _cache["nc"]

    in_maps = []
    for c in range(NCORES):
        sl = slice(c * BL, (c + 1) * BL)
        in_maps.append({"p": P[sl], "q": Q[sl], "s": S[sl], "w": wq, "bq": bq})
    res = run_bass_kernel_spmd(nc, in_maps, list(range(NCORES)))
    _cache["last"] = res  # exec_time_ns etc. when tracing is enabled
    out = np.concatenate([res.results[c]["o"] for c in range(NCORES)], axis=0)
    return np.ascontiguousarray(out.astype(np.float32))
